# revision 2
# baseline (speedup 1.0000x reference)
"""BiLSTM-CRF token-mean NLL loss on 8 Trainium2 NeuronCores.

Sharding: 8 cores = 2 LSTM directions x 4 batch-quarters (B_l=16).
Each core runs: input projection (x @ W_ih^T + b), the 512-step LSTM
recurrence for its direction, and its direction's half of the emission
projection. Host merges the two emission halves per batch-quarter and
computes the (tiny) CRF forward algorithm + gold-path score reduction.

v2 design notes (vs the v1 baseline at 3.81 ms):
  - input-proj xp is stored bf16 in DRAM with an m-major layout so the
    phase-1 writes are contiguous and the recurrence loads one 4KB/
    partition tile per 8-step body (prefetched; v1 stalled ~1.8us/step
    on a late per-step strided load).
  - xp is injected into PSUM with an identity-stationary matmul
    (start=True) so the W_hh matmuls accumulate on top of it; the two
    per-step DVE adds (gate preact = ps + xs) disappear and the sigmoid
    reads PSUM directly.
  - separate PSUM tiles for the i,f,g block and the o block remove a
    false WAR dependency (v1's o-gate matmuls waited on the gate add).
  - h lives entirely in SBUF ([128, (S+1)*64] bf16 ring, slot 0 = h_-1
    = 0); no per-step h DMA; emission matmuls read it directly.
  - all nonlinearities are Sigmoid (tanh(x) = 2*sigmoid(2x) - 1); the
    device stores h' = h/2 and the host doubles W_hh and w_out to
    compensate, so a single activation-function table serves the whole
    loop. Gate math is fused into 3 scalar_tensor_tensor ops + one
    parallel GpSimd multiply:
       u = (sig(2g) - 0.5) * sig(i)         [DVE]   (= i*tanh(g)/2)
       v = sig(f) * c_old                   [GpSimd]
       c = 2*u + v                          [DVE]
       h' = (sig(2c) - 0.5) * sig(o)        [DVE]   (= h/2)

Device layouts (per core):
  xT      [768, 8192] bf16   col p = l*16+b, l = processing step (bwd cores
                             get time-reversed x so the device program is SPMD)
  wih_t   [128, 6*16*128]    stationary tiles (kc, m) of W_ih^T, g-block rows x2
  whh_t   [128, 4*16*128]    stationary tiles (k, m) of W_hh^T, g-rows x2, all x2
  bias    [128, 16] fp32     per-gate-tile bias (g-block x2)
  wo_t    [128, 4*9] bf16    stationary tiles of 2*w_out (this dir's 512 hid cols)
  bias_o  [9, 1] fp32        b_out on fwd cores, 0 on bwd cores
  ident   [128, 128] bf16    identity (stationary for the xp->PSUM inject)
  out: emisT [9, 8192] fp32  emission partial, col p = l*16+b (processing order)
"""

import numpy as np
import ml_dtypes

B, S, EMB = 64, 512, 768
HID = 512
NTAG = 9
BL = 16            # batch per core
NPOS = S * BL      # positions per core
KC_E = EMB // 128  # 6 k-chunks for projection
KC_H = HID // 128  # 4 k-chunks for recurrence
MT = 16            # gate tiles (4*HID/128)
UNROLL = 8

_CACHED = {}


def _build_neff1():
    import concourse.bass as bass
    import concourse.bacc as bacc
    import concourse.mybir as mybir
    import concourse.tile as tile
    from concourse.bass import ds

    f32 = mybir.dt.float32
    bf16 = mybir.dt.bfloat16

    nc = bacc.Bacc("TRN2", target_bir_lowering=False, debug=False)

    xT = nc.dram_tensor("xT", [EMB, NPOS], bf16, kind="ExternalInput")
    wih = nc.dram_tensor("wih", [128, KC_E * MT * 128], bf16, kind="ExternalInput")
    whh = nc.dram_tensor("whh", [128, KC_H * MT * 128], bf16, kind="ExternalInput")
    bias = nc.dram_tensor("bias", [128, MT], f32, kind="ExternalInput")
    wo = nc.dram_tensor("wo", [128, KC_H * NTAG], bf16, kind="ExternalInput")
    bias_o = nc.dram_tensor("bias_o", [NTAG, 1], f32, kind="ExternalInput")
    ident = nc.dram_tensor("ident", [128, 128], bf16, kind="ExternalInput")
    emisT = nc.dram_tensor("emisT", [NTAG, NPOS], f32, kind="ExternalOutput")

    # xp in DRAM, bf16, m-major: col = m*(S*BL) + s*BL + b
    xpT = nc.dram_tensor("xpT", [128, MT * S * BL], bf16)

    sig = mybir.ActivationFunctionType.Sigmoid
    mult = mybir.AluOpType.mult
    add = mybir.AluOpType.add
    subtract = mybir.AluOpType.subtract

    HB = KC_H * BL       # 64 cols: one gate-block (4 hid-chunks x 16 batch)
    BW = UNROLL * BL     # 128 cols per m-tile per body

    with tile.TileContext(nc) as tc:
        with (
            tc.tile_pool(name="wpool", bufs=1) as wpool,
            tc.tile_pool(name="x6pool", bufs=2) as x6pool,
            tc.tile_pool(name="xopool", bufs=3) as xopool,
            tc.tile_pool(name="xbpool", bufs=3) as xbpool,
            tc.tile_pool(name="gpool", bufs=3) as gpool,
            tc.tile_pool(name="spool", bufs=3) as spool,
            tc.tile_pool(name="eopool", bufs=3) as eopool,
            tc.tile_pool(name="pp1", bufs=2, space="PSUM") as pp1,
            tc.tile_pool(name="ppA", bufs=2, space="PSUM") as ppA,
            tc.tile_pool(name="ppB", bufs=2, space="PSUM") as ppB,
            tc.tile_pool(name="pp9", bufs=2, space="PSUM") as pp9,
        ):
            # --- resident weights ---
            wih_sb = wpool.tile([128, KC_E * MT * 128], bf16, tag="wih")
            whh_sb = wpool.tile([128, KC_H * MT * 128], bf16, tag="whh")
            bias_sb = wpool.tile([128, MT], f32, tag="bias")
            wo_sb = wpool.tile([128, KC_H * NTAG], bf16, tag="wo")
            bias_o_sb = wpool.tile([NTAG, 1], f32, tag="biaso")
            ident_sb = wpool.tile([128, 128], bf16, tag="ident")
            nc.sync.dma_start(out=wih_sb[:], in_=wih[:])
            nc.sync.dma_start(out=whh_sb[:], in_=whh[:])
            nc.sync.dma_start(out=bias_sb[:], in_=bias[:])
            nc.sync.dma_start(out=wo_sb[:], in_=wo[:])
            nc.sync.dma_start(out=bias_o_sb[:], in_=bias_o[:])
            nc.sync.dma_start(out=ident_sb[:], in_=ident[:])

            # --- persistent state ---
            # h ring: slot s+1 holds h'(s) = h(s)/2; slot 0 = zeros.
            h_all = nc.alloc_sbuf_tensor("h_all", [128, (S + 1) * HB], bf16).ap()
            c_sb = nc.alloc_sbuf_tensor("c_state", [128, 2 * HB], f32).ap()
            nc.vector.memset(h_all[:, 0:HB], 0.0)
            nc.vector.memset(c_sb[:], 0.0)

            # --- phase 1: input projection -> xpT (bf16, m-major) ---
            for pc in range(MT):
                xs6 = x6pool.tile([128, KC_E * 512], bf16, tag="xs6")
                for kc in range(KC_E):
                    nc.sync.dma_start(
                        out=xs6[:, kc * 512:(kc + 1) * 512],
                        in_=xT[kc * 128:(kc + 1) * 128, pc * 512:(pc + 1) * 512],
                    )
                for m in range(MT):
                    ps = pp1.tile([128, 512], f32, tag="ppj")
                    for kc in range(KC_E):
                        nc.tensor.matmul(
                            ps[:],
                            wih_sb[:, (kc * MT + m) * 128:(kc * MT + m) * 128 + 128],
                            xs6[:, kc * 512:(kc + 1) * 512],
                            start=(kc == 0),
                            stop=(kc == KC_E - 1),
                        )
                    xo = xopool.tile([128, 512], bf16, tag="xo")
                    nc.vector.tensor_scalar_add(xo[:], ps[:], bias_sb[:, m:m + 1])
                    nc.sync.dma_start(
                        out=xpT[:, ds(m * (S * BL) + pc * 512, 512)], in_=xo[:])

            # --- phase 2: recurrence ---
            xpT_m = xpT[:].rearrange("p (m sb) -> p m sb", m=MT)

            def step_body(xbr, iv, i, pi):
                po = 1 - pi
                psA = ppA.tile([128, 12 * BL], f32, tag="psA")
                psB = ppB.tile([128, 4 * BL], f32, tag="psB")

                # inject xp for i,f,g tiles; accumulate W_hh·h'(t-1)
                nc.tensor.matmul(
                    psA[:], ident_sb[:], xbr[:, 0:12, i * BL:(i + 1) * BL],
                    start=True, stop=False)
                for m in range(12):
                    for k in range(KC_H):
                        nc.tensor.matmul(
                            psA[:, m * BL:(m + 1) * BL],
                            whh_sb[:, (k * MT + m) * 128:(k * MT + m) * 128 + 128],
                            h_all[:, ds(iv * HB + k * BL, BL)],
                            start=False,
                            stop=(m == 11 and k == KC_H - 1),
                        )
                # o tiles in their own PSUM bank (no WAR with the sigmoid read)
                nc.tensor.matmul(
                    psB[:], ident_sb[:], xbr[:, 12:16, i * BL:(i + 1) * BL],
                    start=True, stop=False)
                for m in range(12, 16):
                    for k in range(KC_H):
                        nc.tensor.matmul(
                            psB[:, (m - 12) * BL:(m - 11) * BL],
                            whh_sb[:, (k * MT + m) * 128:(k * MT + m) * 128 + 128],
                            h_all[:, ds(iv * HB + k * BL, BL)],
                            start=False,
                            stop=(m == 15 and k == KC_H - 1),
                        )

                G = gpool.tile([128, 12 * BL], bf16, tag="G")
                nc.scalar.activation(G[:], psA[:], sig)
                Go = gpool.tile([128, HB], bf16, tag="Go")
                nc.scalar.activation(Go[:], psB[:], sig)

                # u = (sig2g - 0.5) * sigi ; v = sigf * c_old ; c = 2u + v
                u = spool.tile([128, HB], f32, tag="u")
                nc.vector.scalar_tensor_tensor(
                    u[:], G[:, 8 * BL:12 * BL], 0.5, G[:, 0:4 * BL],
                    subtract, mult)
                v = spool.tile([128, HB], f32, tag="v")
                nc.gpsimd.tensor_tensor(
                    out=v[:], in0=G[:, 4 * BL:8 * BL],
                    in1=c_sb[:, pi * HB:(pi + 1) * HB], op=mult)
                c_new = c_sb[:, po * HB:(po + 1) * HB]
                nc.vector.scalar_tensor_tensor(c_new, u[:], 2.0, v[:], mult, add)
                sc = gpool.tile([128, HB], bf16, tag="sc")
                nc.scalar.activation(sc[:], c_new, sig, scale=2.0)
                # h' = (sig(2c) - 0.5) * sig(o) = h/2
                nc.vector.scalar_tensor_tensor(
                    h_all[:, ds((iv + 1) * HB, HB)], sc[:], 0.5, Go[:],
                    subtract, mult)

            def unrolled(iv0, unroll):
                xb = xbpool.tile([128, MT * BW], bf16, tag="xb")
                xbr = xb[:].rearrange("p (m ib) -> p m ib", m=MT)
                nc.sync.dma_start(
                    out=xbr[:, :, 0:unroll * BL],
                    in_=xpT_m[:, :, ds(iv0 * BL, unroll * BL)],
                )
                for i in range(unroll):
                    step_body(xbr, iv0 + i, i, i % 2)

            tc.For_i_unrolled_general(
                0, S, 1, unrolled, max_unroll=UNROLL,
                hint_engines=(mybir.EngineType.PE, mybir.EngineType.DVE,
                              mybir.EngineType.Activation, mybir.EngineType.Pool,
                              mybir.EngineType.SP),
            )

            # --- phase 3: emissions from SBUF h ring ---
            h_skb = h_all.rearrange("p (s k b) -> p s k b", k=KC_H, b=BL)
            for pc in range(MT):
                ps9 = pp9.tile([NTAG, 512], f32, tag="ps9")
                for kc in range(KC_H):
                    nc.tensor.matmul(
                        ps9[:],
                        wo_sb[:, kc * NTAG:(kc + 1) * NTAG],
                        h_skb[:, ds(pc * 32 + 1, 32), kc, :],
                        start=(kc == 0),
                        stop=(kc == KC_H - 1),
                    )
                eo = eopool.tile([NTAG, 512], f32, tag="eo")
                nc.vector.tensor_scalar_add(eo[:], ps9[:], bias_o_sb[:, 0:1])
                nc.sync.dma_start(out=emisT[:, pc * 512:(pc + 1) * 512], in_=eo[:])

    nc.compile()
    return nc


def _prep_core_inputs(x, w_ih, w_hh, b_all, w_out, b_out, D, q):
    """Build the input dict for core (direction D, batch-quarter q)."""
    bf16 = ml_dtypes.bfloat16
    bs = slice(BL * q, BL * q + BL)
    xs = x[bs]                       # [16, S, EMB]
    if D == 1:
        xs = xs[:, ::-1, :]          # processing order = reversed time
    # xT[e, l*16+b] = xs[b, l, e]
    xT = np.ascontiguousarray(xs.transpose(2, 1, 0).reshape(EMB, NPOS)).astype(bf16)

    gscale = np.ones((4 * HID,), np.float32)
    gscale[2 * HID:3 * HID] = 2.0    # pytorch gate order i,f,g,o -> g block

    wihs = (w_ih * gscale[:, None]).astype(np.float32)   # [2048, 768]
    # extra x2: device stores h' = h/2
    whhs = (w_hh * gscale[:, None] * 2.0).astype(np.float32)   # [2048, 512]
    bs_ = (b_all * gscale).astype(np.float32)            # [2048]

    # wih tiles: [kr, (kc*MT+m)*128+mc] = wihs[m*128+mc, kc*128+kr]
    wt = wihs.reshape(MT, 128, KC_E, 128).transpose(3, 2, 0, 1)   # [kr, kc, m, mc]
    wih_t = np.ascontiguousarray(wt.reshape(128, KC_E * MT * 128)).astype(bf16)
    ht = whhs.reshape(MT, 128, KC_H, 128).transpose(3, 2, 0, 1)
    whh_t = np.ascontiguousarray(ht.reshape(128, KC_H * MT * 128)).astype(bf16)
    bias_t = np.ascontiguousarray(bs_.reshape(MT, 128).T).astype(np.float32)

    # wo tiles: [kr, kc*9+t] = 2*w_out[t, D*512 + kc*128 + kr]  (h' = h/2)
    wo_half = 2.0 * w_out[:, D * HID:(D + 1) * HID]      # [9, 512]
    wo_t = np.ascontiguousarray(
        wo_half.reshape(NTAG, KC_H, 128).transpose(2, 1, 0).reshape(128, KC_H * NTAG)
    ).astype(bf16)
    bias_o = (b_out.reshape(NTAG, 1) if D == 0 else np.zeros((NTAG, 1))).astype(np.float32)
    ident_t = np.eye(128, dtype=np.float32).astype(bf16)

    return {
        "xT": np.asarray(xT), "wih": wih_t, "whh": whh_t, "bias": bias_t,
        "wo": wo_t, "bias_o": bias_o, "ident": ident_t,
    }


def _crf_loss_host(emis, tags, mask, start_trans, end_trans, trans):
    """emis [S, B, T] fp32 (time-major), tags [S, B], mask [S, B]. Exact numpy CRF."""
    Sq, Bq, T = emis.shape
    bidx = np.arange(Bq)
    m = mask.astype(np.float64)
    e = emis.astype(np.float64)
    tr = trans.astype(np.float64)
    num = start_trans.astype(np.float64)[tags[0]] + e[0, bidx, tags[0]]
    trans_steps = tr[tags[:-1], tags[1:]]
    emit_steps = np.take_along_axis(e[1:], tags[1:, :, None], axis=2)[..., 0]
    num = num + ((trans_steps + emit_steps) * m[1:]).sum(0)
    last_idx = m.sum(0).astype(np.int64) - 1
    num = num + end_trans.astype(np.float64)[tags[last_idx, bidx]]

    alpha = start_trans.astype(np.float64) + e[0]        # [B, T]
    for t in range(1, Sq):
        x = alpha[:, :, None] + tr[None] + e[t][:, None, :]
        mx = x.max(1)
        nxt = mx + np.log(np.exp(x - mx[:, None, :]).sum(1))
        alpha = np.where(m[t][:, None] > 0, nxt, alpha)
    z = alpha + end_trans.astype(np.float64)
    mz = z.max(1)
    den = mz + np.log(np.exp(z - mz[:, None]).sum(1))
    llh = num - den
    return -(llh.sum() / m.sum())


def kernel(x, mask, target_tag, w_ih_f, w_hh_f, b_f, w_ih_b, w_hh_b, b_b,
           w_out, b_out, start_trans, end_trans, trans):
    from concourse.bass_utils import run_bass_kernel_spmd

    x = np.asarray(x, np.float32)
    mask = np.asarray(mask)
    target_tag = np.asarray(target_tag)
    w_out = np.asarray(w_out, np.float32)
    b_out = np.asarray(b_out, np.float32)

    if "nc" not in _CACHED:
        _CACHED["nc"] = _build_neff1()
    nc = _CACHED["nc"]

    in_maps = []
    for core in range(8):
        D, q = core // 4, core % 4
        w_ih = np.asarray(w_ih_f if D == 0 else w_ih_b, np.float32)
        w_hh = np.asarray(w_hh_f if D == 0 else w_hh_b, np.float32)
        b_all = np.asarray(b_f if D == 0 else b_b, np.float32)
        in_maps.append(_prep_core_inputs(x, w_ih, w_hh, b_all, w_out, b_out, D, q))

    res = run_bass_kernel_spmd(nc, in_maps, core_ids=list(range(8)))

    # merge emissions: emis[s, b, t]
    emis = np.zeros((S, B, NTAG), np.float32)
    for core in range(8):
        D, q = core // 4, core % 4
        eT = res.results[core]["emisT"]                 # [9, S*16] processing order
        e = eT.reshape(NTAG, S, BL).transpose(1, 2, 0)  # [S(proc), 16, 9]
        if D == 1:
            e = e[::-1]
        emis[:, BL * q:BL * q + BL, :] += e

    loss = _crf_loss_host(
        emis, np.asarray(target_tag).T, np.asarray(mask).T.astype(np.float32),
        np.asarray(start_trans, np.float32), np.asarray(end_trans, np.float32),
        np.asarray(trans, np.float32),
    )
    return np.float32(loss)


# revision 6
# speedup vs baseline: 2.4955x; 2.4955x over previous
"""BiLSTM-CRF token-mean NLL loss on 8 Trainium2 NeuronCores.

Sharding: 8 cores = 2 LSTM directions x 4 batch-quarters (B_l=16).
Each core runs: input projection (x @ W_ih^T + b), the 512-step LSTM
recurrence for its direction, and its direction's half of the emission
projection. Host merges the two emission halves per batch-quarter and
computes the (tiny) CRF forward algorithm + gold-path score reduction.

v2 design notes (vs the v1 baseline at 3.81 ms):
  - input-proj xp is stored bf16 in DRAM with an m-major layout so the
    phase-1 writes are contiguous and the recurrence loads one 4KB/
    partition tile per 8-step body (prefetched; v1 stalled ~1.8us/step
    on a late per-step strided load).
  - xp is injected into PSUM with an identity-stationary matmul
    (start=True) so the W_hh matmuls accumulate on top of it; the two
    per-step DVE adds (gate preact = ps + xs) disappear and the sigmoid
    reads PSUM directly.
  - separate PSUM tiles for the i,f,g block and the o block remove a
    false WAR dependency (v1's o-gate matmuls waited on the gate add).
  - h lives entirely in SBUF ([128, (S+1)*64] bf16 ring, slot 0 = h_-1
    = 0); no per-step h DMA; emission matmuls read it directly.
  - all nonlinearities are Sigmoid (tanh(x) = 2*sigmoid(2x) - 1); the
    device stores h' = h/2 and the host doubles W_hh and w_out to
    compensate, so a single activation-function table serves the whole
    loop. Gate math is fused into 3 scalar_tensor_tensor ops + one
    parallel GpSimd multiply:
       u = (sig(2g) - 0.5) * sig(i)         [DVE]   (= i*tanh(g)/2)
       v = sig(f) * c_old                   [GpSimd]
       c = 2*u + v                          [DVE]
       h' = (sig(2c) - 0.5) * sig(o)        [DVE]   (= h/2)

Device layouts (per core):
  xT      [768, 8192] bf16   col p = l*16+b, l = processing step (bwd cores
                             get time-reversed x so the device program is SPMD)
  wih_t   [128, 6*16*128]    stationary tiles (kc, m) of W_ih^T, g-block rows x2
  whh_t   [128, 4*16*128]    stationary tiles (k, m) of W_hh^T, g-rows x2, all x2
  bias    [128, 16] fp32     per-gate-tile bias (g-block x2)
  wo_t    [128, 4*9] bf16    stationary tiles of 2*w_out (this dir's 512 hid cols)
  bias_o  [9, 1] fp32        b_out on fwd cores, 0 on bwd cores
  ident   [128, 128] bf16    identity (stationary for the xp->PSUM inject)
  out: emisT [9, 8192] fp32  emission partial, col p = l*16+b (processing order)
"""

import numpy as np
import ml_dtypes

B, S, EMB = 64, 512, 768
HID = 512
NTAG = 9
BL = 16            # batch per core
NPOS = S * BL      # positions per core
KC_E = EMB // 128  # 6 k-chunks for projection
KC_H = HID // 128  # 4 k-chunks for recurrence
MT = 16            # gate tiles (4*HID/128)
UNROLL = 8

_CACHED = {}


def _build_neff1():
    import concourse.bass as bass
    import concourse.bacc as bacc
    import concourse.mybir as mybir
    import concourse.tile as tile
    from concourse.bass import ds

    f32 = mybir.dt.float32
    bf16 = mybir.dt.bfloat16

    nc = bacc.Bacc("TRN2", target_bir_lowering=False, debug=False)

    xT = nc.dram_tensor("xT", [EMB, NPOS], bf16, kind="ExternalInput")
    wih = nc.dram_tensor("wih", [128, KC_E * MT * 128], bf16, kind="ExternalInput")
    whh = nc.dram_tensor("whh", [128, KC_H * MT * 128], bf16, kind="ExternalInput")
    bias = nc.dram_tensor("bias", [128, MT], f32, kind="ExternalInput")
    wo = nc.dram_tensor("wo", [128, KC_H * NTAG], bf16, kind="ExternalInput")
    bias_o = nc.dram_tensor("bias_o", [NTAG, 1], f32, kind="ExternalInput")
    ident = nc.dram_tensor("ident", [128, 128], bf16, kind="ExternalInput")
    emisT = nc.dram_tensor("emisT", [NTAG, NPOS], f32, kind="ExternalOutput")

    # xp in DRAM, bf16, m-major: col = m*(S*BL) + s*BL + b
    xpT = nc.dram_tensor("xpT", [128, MT * S * BL], bf16)

    sig = mybir.ActivationFunctionType.Sigmoid
    mult = mybir.AluOpType.mult
    add = mybir.AluOpType.add
    subtract = mybir.AluOpType.subtract

    HB = KC_H * BL       # 64 cols: one gate-block (4 hid-chunks x 16 batch)
    BW = UNROLL * BL     # 128 cols per m-tile per body

    with tile.TileContext(nc) as tc:
        with (
            tc.tile_pool(name="wpool", bufs=1) as wpool,
            tc.tile_pool(name="x6pool", bufs=2) as x6pool,
            tc.tile_pool(name="xopool", bufs=3) as xopool,
            tc.tile_pool(name="xbpool", bufs=3) as xbpool,
            tc.tile_pool(name="gpool", bufs=3) as gpool,
            tc.tile_pool(name="spool", bufs=3) as spool,
            tc.tile_pool(name="eopool", bufs=3) as eopool,
            tc.tile_pool(name="pp1", bufs=2, space="PSUM") as pp1,
            tc.tile_pool(name="ppA", bufs=2, space="PSUM") as ppA,
            tc.tile_pool(name="ppB", bufs=2, space="PSUM") as ppB,
            tc.tile_pool(name="pp9", bufs=2, space="PSUM") as pp9,
        ):
            # --- resident weights ---
            wih_sb = wpool.tile([128, KC_E * MT * 128], bf16, tag="wih")
            whh_sb = wpool.tile([128, KC_H * MT * 128], bf16, tag="whh")
            bias_sb = wpool.tile([128, MT], f32, tag="bias")
            wo_sb = wpool.tile([128, KC_H * NTAG], bf16, tag="wo")
            bias_o_sb = wpool.tile([NTAG, 1], f32, tag="biaso")
            ident_sb = wpool.tile([128, 128], bf16, tag="ident")
            nc.sync.dma_start(out=wih_sb[:], in_=wih[:])
            nc.sync.dma_start(out=whh_sb[:], in_=whh[:])
            nc.sync.dma_start(out=bias_sb[:], in_=bias[:])
            nc.sync.dma_start(out=wo_sb[:], in_=wo[:])
            nc.sync.dma_start(out=bias_o_sb[:], in_=bias_o[:])
            nc.sync.dma_start(out=ident_sb[:], in_=ident[:])

            # --- persistent state ---
            # h ring (for emissions): slot s holds h'(s) = h(s)/2.
            # h_sb: static ping-pong read by the recurrence matmuls (register
            # offsets on matmul operands cost ~100ns/instruction on the PE
            # sequencer, so the hot loop must use static addresses).
            h_all = nc.alloc_sbuf_tensor("h_all", [128, S * HB], bf16).ap()
            h_sb = nc.alloc_sbuf_tensor("h_state", [128, 2 * HB], bf16).ap()
            c_sb = nc.alloc_sbuf_tensor("c_state", [128, 2 * HB], f32).ap()
            nc.vector.memset(h_sb[:], 0.0)
            nc.vector.memset(c_sb[:], 0.0)

            # --- phase 1: input projection -> xpT (bf16, m-major) ---
            for pc in range(MT):
                xs6 = x6pool.tile([128, KC_E * 512], bf16, tag="xs6")
                for kc in range(KC_E):
                    nc.sync.dma_start(
                        out=xs6[:, kc * 512:(kc + 1) * 512],
                        in_=xT[kc * 128:(kc + 1) * 128, pc * 512:(pc + 1) * 512],
                    )
                for m in range(MT):
                    ps = pp1.tile([128, 512], f32, tag="ppj")
                    for kc in range(KC_E):
                        nc.tensor.matmul(
                            ps[:],
                            wih_sb[:, (kc * MT + m) * 128:(kc * MT + m) * 128 + 128],
                            xs6[:, kc * 512:(kc + 1) * 512],
                            start=(kc == 0),
                            stop=(kc == KC_E - 1),
                        )
                    xo = xopool.tile([128, 512], bf16, tag="xo")
                    nc.vector.tensor_scalar_add(xo[:], ps[:], bias_sb[:, m:m + 1])
                    nc.sync.dma_start(
                        out=xpT[:, ds(m * (S * BL) + pc * 512, 512)], in_=xo[:])

            # --- phase 2: recurrence ---
            xpT_m = xpT[:].rearrange("p (m sb) -> p m sb", m=MT)

            def step_body(xbr, iv, i, pi):
                po = 1 - pi
                psA = ppA.tile([128, 12 * BL], f32, tag="psA")
                psB = ppB.tile([128, 4 * BL], f32, tag="psB")

                # inject xp for i,f,g tiles; accumulate W_hh·h'(t-1)
                nc.tensor.matmul(
                    psA[:], ident_sb[:], xbr[:, 0:12, i * BL:(i + 1) * BL],
                    start=True, stop=False)
                for m in range(12):
                    for k in range(KC_H):
                        nc.tensor.matmul(
                            psA[:, m * BL:(m + 1) * BL],
                            whh_sb[:, (k * MT + m) * 128:(k * MT + m) * 128 + 128],
                            h_sb[:, pi * HB + k * BL:pi * HB + (k + 1) * BL],
                            start=False,
                            stop=(m == 11 and k == KC_H - 1),
                        )
                # o tiles in their own PSUM bank (no WAR with the sigmoid read)
                nc.tensor.matmul(
                    psB[:], ident_sb[:], xbr[:, 12:16, i * BL:(i + 1) * BL],
                    start=True, stop=False)
                for m in range(12, 16):
                    for k in range(KC_H):
                        nc.tensor.matmul(
                            psB[:, (m - 12) * BL:(m - 11) * BL],
                            whh_sb[:, (k * MT + m) * 128:(k * MT + m) * 128 + 128],
                            h_sb[:, pi * HB + k * BL:pi * HB + (k + 1) * BL],
                            start=False,
                            stop=(m == 15 and k == KC_H - 1),
                        )

                G = gpool.tile([128, 12 * BL], bf16, tag="G")
                nc.scalar.activation(G[:], psA[:], sig)
                Go = gpool.tile([128, HB], bf16, tag="Go")
                nc.scalar.activation(Go[:], psB[:], sig)

                # u = (sig2g - 0.5) * sigi ; v = sigf * c_old ; c = 2u + v
                u = spool.tile([128, HB], f32, tag="u")
                nc.vector.scalar_tensor_tensor(
                    u[:], G[:, 8 * BL:12 * BL], 0.5, G[:, 0:4 * BL],
                    subtract, mult)
                v = spool.tile([128, HB], f32, tag="v")
                nc.gpsimd.tensor_tensor(
                    out=v[:], in0=G[:, 4 * BL:8 * BL],
                    in1=c_sb[:, pi * HB:(pi + 1) * HB], op=mult)
                c_new = c_sb[:, po * HB:(po + 1) * HB]
                nc.vector.scalar_tensor_tensor(c_new, u[:], 2.0, v[:], mult, add)
                sc = gpool.tile([128, HB], bf16, tag="sc")
                nc.scalar.activation(sc[:], c_new, sig, scale=2.0)
                # h' = (sig(2c) - 0.5) * sig(o) = h/2
                h_new = h_sb[:, po * HB:(po + 1) * HB]
                nc.vector.scalar_tensor_tensor(
                    h_new, sc[:], 0.5, Go[:], subtract, mult)
                # mirror into the emission ring, off the critical path
                nc.gpsimd.tensor_copy(out=h_all[:, ds(iv * HB, HB)], in_=h_new)

            def unrolled(iv0, unroll):
                xb = xbpool.tile([128, MT * BW], bf16, tag="xb")
                xbr = xb[:].rearrange("p (m ib) -> p m ib", m=MT)
                nc.sync.dma_start(
                    out=xbr[:, :, 0:unroll * BL],
                    in_=xpT_m[:, :, ds(iv0 * BL, unroll * BL)],
                )
                for i in range(unroll):
                    step_body(xbr, iv0 + i, i, i % 2)

            tc.For_i_unrolled_general(
                0, S, 1, unrolled, max_unroll=UNROLL,
                hint_engines=(mybir.EngineType.PE, mybir.EngineType.DVE,
                              mybir.EngineType.Activation, mybir.EngineType.Pool,
                              mybir.EngineType.SP),
            )

            # --- phase 3: emissions from SBUF h ring ---
            h_skb = h_all.rearrange("p (s k b) -> p s k b", k=KC_H, b=BL)
            for pc in range(MT):
                ps9 = pp9.tile([NTAG, 512], f32, tag="ps9")
                for kc in range(KC_H):
                    nc.tensor.matmul(
                        ps9[:],
                        wo_sb[:, kc * NTAG:(kc + 1) * NTAG],
                        h_skb[:, pc * 32:(pc + 1) * 32, kc, :],
                        start=(kc == 0),
                        stop=(kc == KC_H - 1),
                    )
                eo = eopool.tile([NTAG, 512], f32, tag="eo")
                nc.vector.tensor_scalar_add(eo[:], ps9[:], bias_o_sb[:, 0:1])
                nc.sync.dma_start(out=emisT[:, pc * 512:(pc + 1) * 512], in_=eo[:])

    nc.compile()
    return nc


def _prep_core_inputs(x, w_ih, w_hh, b_all, w_out, b_out, D, q):
    """Build the input dict for core (direction D, batch-quarter q)."""
    bf16 = ml_dtypes.bfloat16
    bs = slice(BL * q, BL * q + BL)
    xs = x[bs]                       # [16, S, EMB]
    if D == 1:
        xs = xs[:, ::-1, :]          # processing order = reversed time
    # xT[e, l*16+b] = xs[b, l, e]
    xT = np.ascontiguousarray(xs.transpose(2, 1, 0).reshape(EMB, NPOS)).astype(bf16)

    gscale = np.ones((4 * HID,), np.float32)
    gscale[2 * HID:3 * HID] = 2.0    # pytorch gate order i,f,g,o -> g block

    wihs = (w_ih * gscale[:, None]).astype(np.float32)   # [2048, 768]
    # extra x2: device stores h' = h/2
    whhs = (w_hh * gscale[:, None] * 2.0).astype(np.float32)   # [2048, 512]
    bs_ = (b_all * gscale).astype(np.float32)            # [2048]

    # wih tiles: [kr, (kc*MT+m)*128+mc] = wihs[m*128+mc, kc*128+kr]
    wt = wihs.reshape(MT, 128, KC_E, 128).transpose(3, 2, 0, 1)   # [kr, kc, m, mc]
    wih_t = np.ascontiguousarray(wt.reshape(128, KC_E * MT * 128)).astype(bf16)
    ht = whhs.reshape(MT, 128, KC_H, 128).transpose(3, 2, 0, 1)
    whh_t = np.ascontiguousarray(ht.reshape(128, KC_H * MT * 128)).astype(bf16)
    bias_t = np.ascontiguousarray(bs_.reshape(MT, 128).T).astype(np.float32)

    # wo tiles: [kr, kc*9+t] = 2*w_out[t, D*512 + kc*128 + kr]  (h' = h/2)
    wo_half = 2.0 * w_out[:, D * HID:(D + 1) * HID]      # [9, 512]
    wo_t = np.ascontiguousarray(
        wo_half.reshape(NTAG, KC_H, 128).transpose(2, 1, 0).reshape(128, KC_H * NTAG)
    ).astype(bf16)
    bias_o = (b_out.reshape(NTAG, 1) if D == 0 else np.zeros((NTAG, 1))).astype(np.float32)
    ident_t = np.eye(128, dtype=np.float32).astype(bf16)

    return {
        "xT": np.asarray(xT), "wih": wih_t, "whh": whh_t, "bias": bias_t,
        "wo": wo_t, "bias_o": bias_o, "ident": ident_t,
    }


def _crf_loss_host(emis, tags, mask, start_trans, end_trans, trans):
    """emis [S, B, T] fp32 (time-major), tags [S, B], mask [S, B]. Exact numpy CRF."""
    Sq, Bq, T = emis.shape
    bidx = np.arange(Bq)
    m = mask.astype(np.float64)
    e = emis.astype(np.float64)
    tr = trans.astype(np.float64)
    num = start_trans.astype(np.float64)[tags[0]] + e[0, bidx, tags[0]]
    trans_steps = tr[tags[:-1], tags[1:]]
    emit_steps = np.take_along_axis(e[1:], tags[1:, :, None], axis=2)[..., 0]
    num = num + ((trans_steps + emit_steps) * m[1:]).sum(0)
    last_idx = m.sum(0).astype(np.int64) - 1
    num = num + end_trans.astype(np.float64)[tags[last_idx, bidx]]

    alpha = start_trans.astype(np.float64) + e[0]        # [B, T]
    for t in range(1, Sq):
        x = alpha[:, :, None] + tr[None] + e[t][:, None, :]
        mx = x.max(1)
        nxt = mx + np.log(np.exp(x - mx[:, None, :]).sum(1))
        alpha = np.where(m[t][:, None] > 0, nxt, alpha)
    z = alpha + end_trans.astype(np.float64)
    mz = z.max(1)
    den = mz + np.log(np.exp(z - mz[:, None]).sum(1))
    llh = num - den
    return -(llh.sum() / m.sum())


def kernel(x, mask, target_tag, w_ih_f, w_hh_f, b_f, w_ih_b, w_hh_b, b_b,
           w_out, b_out, start_trans, end_trans, trans):
    from concourse.bass_utils import run_bass_kernel_spmd

    x = np.asarray(x, np.float32)
    mask = np.asarray(mask)
    target_tag = np.asarray(target_tag)
    w_out = np.asarray(w_out, np.float32)
    b_out = np.asarray(b_out, np.float32)

    if "nc" not in _CACHED:
        _CACHED["nc"] = _build_neff1()
    nc = _CACHED["nc"]

    in_maps = []
    for core in range(8):
        D, q = core // 4, core % 4
        w_ih = np.asarray(w_ih_f if D == 0 else w_ih_b, np.float32)
        w_hh = np.asarray(w_hh_f if D == 0 else w_hh_b, np.float32)
        b_all = np.asarray(b_f if D == 0 else b_b, np.float32)
        in_maps.append(_prep_core_inputs(x, w_ih, w_hh, b_all, w_out, b_out, D, q))

    res = run_bass_kernel_spmd(nc, in_maps, core_ids=list(range(8)))

    # merge emissions: emis[s, b, t]
    emis = np.zeros((S, B, NTAG), np.float32)
    for core in range(8):
        D, q = core // 4, core % 4
        eT = res.results[core]["emisT"]                 # [9, S*16] processing order
        e = eT.reshape(NTAG, S, BL).transpose(1, 2, 0)  # [S(proc), 16, 9]
        if D == 1:
            e = e[::-1]
        emis[:, BL * q:BL * q + BL, :] += e

    loss = _crf_loss_host(
        emis, np.asarray(target_tag).T, np.asarray(mask).T.astype(np.float32),
        np.asarray(start_trans, np.float32), np.asarray(end_trans, np.float32),
        np.asarray(trans, np.float32),
    )
    return np.float32(loss)


# revision 10
# speedup vs baseline: 2.6571x; 1.0648x over previous
"""BiLSTM-CRF token-mean NLL loss on 8 Trainium2 NeuronCores.

Sharding: 8 cores = 2 LSTM directions x 4 batch-quarters (B_l=16).
Each core runs: input projection (x @ W_ih^T + b), the 512-step LSTM
recurrence for its direction, and its direction's half of the emission
projection. Host merges the two emission halves per batch-quarter and
computes the (tiny) CRF forward algorithm + gold-path score reduction.

v2 design notes (vs the v1 baseline at 3.81 ms):
  - input-proj xp is stored bf16 in DRAM with an m-major layout so the
    phase-1 writes are contiguous and the recurrence loads one 4KB/
    partition tile per 8-step body (prefetched; v1 stalled ~1.8us/step
    on a late per-step strided load).
  - xp is injected into PSUM with an identity-stationary matmul
    (start=True) so the W_hh matmuls accumulate on top of it; the two
    per-step DVE adds (gate preact = ps + xs) disappear and the sigmoid
    reads PSUM directly.
  - separate PSUM tiles for the i,f,g block and the o block remove a
    false WAR dependency (v1's o-gate matmuls waited on the gate add).
  - h lives entirely in SBUF ([128, (S+1)*64] bf16 ring, slot 0 = h_-1
    = 0); no per-step h DMA; emission matmuls read it directly.
  - all nonlinearities are Sigmoid (tanh(x) = 2*sigmoid(2x) - 1); the
    device stores h' = h/2 and the host doubles W_hh and w_out to
    compensate, so a single activation-function table serves the whole
    loop. Gate math is fused into 3 scalar_tensor_tensor ops + one
    parallel GpSimd multiply:
       u = (sig(2g) - 0.5) * sig(i)         [DVE]   (= i*tanh(g)/2)
       v = sig(f) * c_old                   [GpSimd]
       c = 2*u + v                          [DVE]
       h' = (sig(2c) - 0.5) * sig(o)        [DVE]   (= h/2)

Device layouts (per core):
  xT      [768, 8192] bf16   col p = l*16+b, l = processing step (bwd cores
                             get time-reversed x so the device program is SPMD)
  wih_t   [128, 6*16*128]    stationary tiles (kc, m) of W_ih^T, g-block rows x2
  whh_t   [128, 4*16*128]    stationary tiles (k, m) of W_hh^T, g-rows x2, all x2
  bias    [128, 16] fp32     per-gate-tile bias (g-block x2)
  wo_t    [128, 4*9] bf16    stationary tiles of 2*w_out (this dir's 512 hid cols)
  bias_o  [9, 1] fp32        b_out on fwd cores, 0 on bwd cores
  ident   [128, 128] bf16    identity (stationary for the xp->PSUM inject)
  out: emisT [9, 8192] fp32  emission partial, col p = l*16+b (processing order)
"""

import numpy as np
import ml_dtypes

B, S, EMB = 64, 512, 768
HID = 512
NTAG = 9
BL = 16            # batch per core
NPOS = S * BL      # positions per core
KC_E = EMB // 128  # 6 k-chunks for projection
KC_H = HID // 128  # 4 k-chunks for recurrence
MT = 16            # gate tiles (4*HID/128)
UNROLL = 16
WHH_FP8 = True     # W_hh stationary in fp8e4m3: FWL loads 4 elem/cycle vs 2

_CACHED = {}


def _build_neff1():
    import concourse.bass as bass
    import concourse.bacc as bacc
    import concourse.mybir as mybir
    import concourse.tile as tile
    from concourse.bass import ds

    f32 = mybir.dt.float32
    bf16 = mybir.dt.bfloat16
    whh_dt = mybir.dt.float8e4 if WHH_FP8 else bf16

    nc = bacc.Bacc("TRN2", target_bir_lowering=False, debug=False)

    xT = nc.dram_tensor("xT", [EMB, NPOS], bf16, kind="ExternalInput")
    wih = nc.dram_tensor("wih", [128, KC_E * MT * 128], bf16, kind="ExternalInput")
    whh = nc.dram_tensor("whh", [128, KC_H * MT * 128], whh_dt, kind="ExternalInput")
    bias = nc.dram_tensor("bias", [128, MT], f32, kind="ExternalInput")
    wo = nc.dram_tensor("wo", [128, KC_H * NTAG], bf16, kind="ExternalInput")
    bias_o = nc.dram_tensor("bias_o", [NTAG, 1], f32, kind="ExternalInput")
    ident = nc.dram_tensor("ident", [128, 128], bf16, kind="ExternalInput")
    emisT = nc.dram_tensor("emisT", [NTAG, NPOS], f32, kind="ExternalOutput")

    # xp in DRAM, bf16, m-major: col = m*(S*BL) + s*BL + b
    xpT = nc.dram_tensor("xpT", [128, MT * S * BL], bf16)

    sig = mybir.ActivationFunctionType.Sigmoid
    mult = mybir.AluOpType.mult
    add = mybir.AluOpType.add
    subtract = mybir.AluOpType.subtract

    HB = KC_H * BL       # 64 cols: one gate-block (4 hid-chunks x 16 batch)
    BW = UNROLL * BL     # 128 cols per m-tile per body

    with tile.TileContext(nc) as tc:
        with (
            tc.tile_pool(name="wpool", bufs=1) as wpool,
            tc.tile_pool(name="x6pool", bufs=2) as x6pool,
            tc.tile_pool(name="xopool", bufs=3) as xopool,
            tc.tile_pool(name="xbpool", bufs=3) as xbpool,
            tc.tile_pool(name="gpool", bufs=3) as gpool,
            tc.tile_pool(name="spool", bufs=3) as spool,
            tc.tile_pool(name="eopool", bufs=3) as eopool,
            tc.tile_pool(name="pp1", bufs=2, space="PSUM") as pp1,
            tc.tile_pool(name="ppA", bufs=2, space="PSUM") as ppA,
            tc.tile_pool(name="ppB", bufs=2, space="PSUM") as ppB,
            tc.tile_pool(name="pp9", bufs=2, space="PSUM") as pp9,
        ):
            # --- resident weights ---
            wih_sb = wpool.tile([128, KC_E * MT * 128], bf16, tag="wih")
            whh_sb = wpool.tile([128, KC_H * MT * 128], whh_dt, tag="whh")
            bias_sb = wpool.tile([128, MT], f32, tag="bias")
            wo_sb = wpool.tile([128, KC_H * NTAG], bf16, tag="wo")
            bias_o_sb = wpool.tile([NTAG, 1], f32, tag="biaso")
            ident_sb = wpool.tile([128, 128], bf16, tag="ident")
            nc.sync.dma_start(out=wih_sb[:], in_=wih[:])
            nc.sync.dma_start(out=whh_sb[:], in_=whh[:])
            nc.sync.dma_start(out=bias_sb[:], in_=bias[:])
            nc.sync.dma_start(out=wo_sb[:], in_=wo[:])
            nc.sync.dma_start(out=bias_o_sb[:], in_=bias_o[:])
            nc.sync.dma_start(out=ident_sb[:], in_=ident[:])

            # --- persistent state ---
            # h ring (for emissions): slot s holds h'(s) = h(s)/2.
            # h_sb: static ping-pong read by the recurrence matmuls (register
            # offsets on matmul operands cost ~100ns/instruction on the PE
            # sequencer, so the hot loop must use static addresses).
            h_all = nc.alloc_sbuf_tensor("h_all", [128, S * HB], bf16).ap()
            h_sb = nc.alloc_sbuf_tensor("h_state", [128, 2 * HB], bf16).ap()
            c_sb = nc.alloc_sbuf_tensor("c_state", [128, 2 * HB], f32).ap()
            nc.vector.memset(h_sb[:], 0.0)
            nc.vector.memset(c_sb[:], 0.0)

            # --- phase 1: input projection -> xpT (bf16, m-major) ---
            for pc in range(MT):
                xs6 = x6pool.tile([128, KC_E * 512], bf16, tag="xs6")
                for kc in range(KC_E):
                    nc.sync.dma_start(
                        out=xs6[:, kc * 512:(kc + 1) * 512],
                        in_=xT[kc * 128:(kc + 1) * 128, pc * 512:(pc + 1) * 512],
                    )
                for m in range(MT):
                    ps = pp1.tile([128, 512], f32, tag="ppj")
                    for kc in range(KC_E):
                        nc.tensor.matmul(
                            ps[:],
                            wih_sb[:, (kc * MT + m) * 128:(kc * MT + m) * 128 + 128],
                            xs6[:, kc * 512:(kc + 1) * 512],
                            start=(kc == 0),
                            stop=(kc == KC_E - 1),
                        )
                    xo = xopool.tile([128, 512], bf16, tag="xo")
                    nc.vector.tensor_scalar_add(xo[:], ps[:], bias_sb[:, m:m + 1])
                    nc.sync.dma_start(
                        out=xpT[:, ds(m * (S * BL) + pc * 512, 512)], in_=xo[:])

            # --- phase 2: recurrence ---
            xpT_m = xpT[:].rearrange("p (m sb) -> p m sb", m=MT)

            def step_body(xbr, iv, i, pi):
                po = 1 - pi
                psA = ppA.tile([128, 12 * BL], f32, tag="psA")
                psB = ppB.tile([128, 4 * BL], f32, tag="psB")

                # inject xp for i,f,g tiles; accumulate W_hh·h'(t-1)
                nc.tensor.matmul(
                    psA[:], ident_sb[:], xbr[:, 0:12, i * BL:(i + 1) * BL],
                    start=True, stop=False)
                for m in range(12):
                    for k in range(KC_H):
                        nc.tensor.matmul(
                            psA[:, m * BL:(m + 1) * BL],
                            whh_sb[:, (k * MT + m) * 128:(k * MT + m) * 128 + 128],
                            h_sb[:, pi * HB + k * BL:pi * HB + (k + 1) * BL],
                            start=False,
                            stop=(m == 11 and k == KC_H - 1),
                        )
                # o tiles in their own PSUM bank (no WAR with the sigmoid read)
                nc.tensor.matmul(
                    psB[:], ident_sb[:], xbr[:, 12:16, i * BL:(i + 1) * BL],
                    start=True, stop=False)
                for m in range(12, 16):
                    for k in range(KC_H):
                        nc.tensor.matmul(
                            psB[:, (m - 12) * BL:(m - 11) * BL],
                            whh_sb[:, (k * MT + m) * 128:(k * MT + m) * 128 + 128],
                            h_sb[:, pi * HB + k * BL:pi * HB + (k + 1) * BL],
                            start=False,
                            stop=(m == 15 and k == KC_H - 1),
                        )

                G = gpool.tile([128, 12 * BL], bf16, tag="G")
                nc.scalar.activation(G[:], psA[:], sig)
                Go = gpool.tile([128, HB], bf16, tag="Go")
                nc.scalar.activation(Go[:], psB[:], sig)

                # u = (sig2g - 0.5) * sigi ; v = sigf * c_old ; c = 2u + v
                u = spool.tile([128, HB], f32, tag="u")
                nc.vector.scalar_tensor_tensor(
                    u[:], G[:, 8 * BL:12 * BL], 0.5, G[:, 0:4 * BL],
                    subtract, mult)
                v = spool.tile([128, HB], f32, tag="v")
                nc.gpsimd.tensor_tensor(
                    out=v[:], in0=G[:, 4 * BL:8 * BL],
                    in1=c_sb[:, pi * HB:(pi + 1) * HB], op=mult)
                c_new = c_sb[:, po * HB:(po + 1) * HB]
                nc.vector.scalar_tensor_tensor(c_new, u[:], 2.0, v[:], mult, add)
                sc = gpool.tile([128, HB], bf16, tag="sc")
                nc.scalar.activation(sc[:], c_new, sig, scale=2.0)
                # h' = (sig(2c) - 0.5) * sig(o) = h/2
                h_new = h_sb[:, po * HB:(po + 1) * HB]
                nc.vector.scalar_tensor_tensor(
                    h_new, sc[:], 0.5, Go[:], subtract, mult)
                # mirror into the emission ring, off the critical path
                nc.gpsimd.tensor_copy(out=h_all[:, ds(iv * HB, HB)], in_=h_new)

            def unrolled(iv0, unroll):
                xb = xbpool.tile([128, MT * BW], bf16, tag="xb")
                xbr = xb[:].rearrange("p (m ib) -> p m ib", m=MT)
                nc.sync.dma_start(
                    out=xbr[:, :, 0:unroll * BL],
                    in_=xpT_m[:, :, ds(iv0 * BL, unroll * BL)],
                )
                for i in range(unroll):
                    step_body(xbr, iv0 + i, i, i % 2)

            tc.For_i_unrolled_general(
                0, S, 1, unrolled, max_unroll=UNROLL,
                hint_engines=(mybir.EngineType.PE, mybir.EngineType.DVE,
                              mybir.EngineType.Activation, mybir.EngineType.Pool,
                              mybir.EngineType.SP),
            )

            # --- phase 3: emissions from SBUF h ring ---
            h_skb = h_all.rearrange("p (s k b) -> p s k b", k=KC_H, b=BL)
            for pc in range(MT):
                ps9 = pp9.tile([NTAG, 512], f32, tag="ps9")
                for kc in range(KC_H):
                    nc.tensor.matmul(
                        ps9[:],
                        wo_sb[:, kc * NTAG:(kc + 1) * NTAG],
                        h_skb[:, pc * 32:(pc + 1) * 32, kc, :],
                        start=(kc == 0),
                        stop=(kc == KC_H - 1),
                    )
                eo = eopool.tile([NTAG, 512], f32, tag="eo")
                nc.vector.tensor_scalar_add(eo[:], ps9[:], bias_o_sb[:, 0:1])
                nc.sync.dma_start(out=emisT[:, pc * 512:(pc + 1) * 512], in_=eo[:])

    nc.compile()
    return nc


def _prep_core_inputs(x, w_ih, w_hh, b_all, w_out, b_out, D, q):
    """Build the input dict for core (direction D, batch-quarter q)."""
    bf16 = ml_dtypes.bfloat16
    bs = slice(BL * q, BL * q + BL)
    xs = x[bs]                       # [16, S, EMB]
    if D == 1:
        xs = xs[:, ::-1, :]          # processing order = reversed time
    # xT[e, l*16+b] = xs[b, l, e]
    xT = np.ascontiguousarray(xs.transpose(2, 1, 0).reshape(EMB, NPOS)).astype(bf16)

    gscale = np.ones((4 * HID,), np.float32)
    gscale[2 * HID:3 * HID] = 2.0    # pytorch gate order i,f,g,o -> g block

    wihs = (w_ih * gscale[:, None]).astype(np.float32)   # [2048, 768]
    # extra x2: device stores h' = h/2
    whhs = (w_hh * gscale[:, None] * 2.0).astype(np.float32)   # [2048, 512]
    bs_ = (b_all * gscale).astype(np.float32)            # [2048]

    # wih tiles: [kr, (kc*MT+m)*128+mc] = wihs[m*128+mc, kc*128+kr]
    wt = wihs.reshape(MT, 128, KC_E, 128).transpose(3, 2, 0, 1)   # [kr, kc, m, mc]
    wih_t = np.ascontiguousarray(wt.reshape(128, KC_E * MT * 128)).astype(bf16)
    ht = whhs.reshape(MT, 128, KC_H, 128).transpose(3, 2, 0, 1)
    whh_np_dt = ml_dtypes.float8_e4m3 if WHH_FP8 else bf16
    whh_t = np.ascontiguousarray(ht.reshape(128, KC_H * MT * 128)).astype(whh_np_dt)
    bias_t = np.ascontiguousarray(bs_.reshape(MT, 128).T).astype(np.float32)

    # wo tiles: [kr, kc*9+t] = 2*w_out[t, D*512 + kc*128 + kr]  (h' = h/2)
    wo_half = 2.0 * w_out[:, D * HID:(D + 1) * HID]      # [9, 512]
    wo_t = np.ascontiguousarray(
        wo_half.reshape(NTAG, KC_H, 128).transpose(2, 1, 0).reshape(128, KC_H * NTAG)
    ).astype(bf16)
    bias_o = (b_out.reshape(NTAG, 1) if D == 0 else np.zeros((NTAG, 1))).astype(np.float32)
    ident_t = np.eye(128, dtype=np.float32).astype(bf16)

    return {
        "xT": np.asarray(xT), "wih": wih_t, "whh": whh_t, "bias": bias_t,
        "wo": wo_t, "bias_o": bias_o, "ident": ident_t,
    }


def _crf_loss_host(emis, tags, mask, start_trans, end_trans, trans):
    """emis [S, B, T] fp32 (time-major), tags [S, B], mask [S, B]. Exact numpy CRF."""
    Sq, Bq, T = emis.shape
    bidx = np.arange(Bq)
    m = mask.astype(np.float64)
    e = emis.astype(np.float64)
    tr = trans.astype(np.float64)
    num = start_trans.astype(np.float64)[tags[0]] + e[0, bidx, tags[0]]
    trans_steps = tr[tags[:-1], tags[1:]]
    emit_steps = np.take_along_axis(e[1:], tags[1:, :, None], axis=2)[..., 0]
    num = num + ((trans_steps + emit_steps) * m[1:]).sum(0)
    last_idx = m.sum(0).astype(np.int64) - 1
    num = num + end_trans.astype(np.float64)[tags[last_idx, bidx]]

    alpha = start_trans.astype(np.float64) + e[0]        # [B, T]
    for t in range(1, Sq):
        x = alpha[:, :, None] + tr[None] + e[t][:, None, :]
        mx = x.max(1)
        nxt = mx + np.log(np.exp(x - mx[:, None, :]).sum(1))
        alpha = np.where(m[t][:, None] > 0, nxt, alpha)
    z = alpha + end_trans.astype(np.float64)
    mz = z.max(1)
    den = mz + np.log(np.exp(z - mz[:, None]).sum(1))
    llh = num - den
    return -(llh.sum() / m.sum())


def kernel(x, mask, target_tag, w_ih_f, w_hh_f, b_f, w_ih_b, w_hh_b, b_b,
           w_out, b_out, start_trans, end_trans, trans):
    from concourse.bass_utils import run_bass_kernel_spmd

    x = np.asarray(x, np.float32)
    mask = np.asarray(mask)
    target_tag = np.asarray(target_tag)
    w_out = np.asarray(w_out, np.float32)
    b_out = np.asarray(b_out, np.float32)

    if "nc" not in _CACHED:
        _CACHED["nc"] = _build_neff1()
    nc = _CACHED["nc"]

    in_maps = []
    for core in range(8):
        D, q = core // 4, core % 4
        w_ih = np.asarray(w_ih_f if D == 0 else w_ih_b, np.float32)
        w_hh = np.asarray(w_hh_f if D == 0 else w_hh_b, np.float32)
        b_all = np.asarray(b_f if D == 0 else b_b, np.float32)
        in_maps.append(_prep_core_inputs(x, w_ih, w_hh, b_all, w_out, b_out, D, q))

    res = run_bass_kernel_spmd(nc, in_maps, core_ids=list(range(8)))

    # merge emissions: emis[s, b, t]
    emis = np.zeros((S, B, NTAG), np.float32)
    for core in range(8):
        D, q = core // 4, core % 4
        eT = res.results[core]["emisT"]                 # [9, S*16] processing order
        e = eT.reshape(NTAG, S, BL).transpose(1, 2, 0)  # [S(proc), 16, 9]
        if D == 1:
            e = e[::-1]
        emis[:, BL * q:BL * q + BL, :] += e

    loss = _crf_loss_host(
        emis, np.asarray(target_tag).T, np.asarray(mask).T.astype(np.float32),
        np.asarray(start_trans, np.float32), np.asarray(end_trans, np.float32),
        np.asarray(trans, np.float32),
    )
    return np.float32(loss)


# revision 14
# speedup vs baseline: 3.2044x; 1.2060x over previous
"""BiLSTM-CRF token-mean NLL loss on 8 Trainium2 NeuronCores.

Sharding: 8 cores = 2 LSTM directions x 4 batch-quarters (B_l=16).
Each core runs: input projection (x @ W_ih^T + b), the 512-step LSTM
recurrence for its direction, and its direction's half of the emission
projection. Host merges the two emission halves per batch-quarter and
computes the (tiny) CRF forward algorithm + gold-path score reduction.

v2 design notes (vs the v1 baseline at 3.81 ms):
  - input-proj xp is stored bf16 in DRAM with an m-major layout so the
    phase-1 writes are contiguous and the recurrence loads one 4KB/
    partition tile per 8-step body (prefetched; v1 stalled ~1.8us/step
    on a late per-step strided load).
  - xp is injected into PSUM with an identity-stationary matmul
    (start=True) so the W_hh matmuls accumulate on top of it; the two
    per-step DVE adds (gate preact = ps + xs) disappear and the sigmoid
    reads PSUM directly.
  - separate PSUM tiles for the i,f,g block and the o block remove a
    false WAR dependency (v1's o-gate matmuls waited on the gate add).
  - h lives entirely in SBUF ([128, (S+1)*64] bf16 ring, slot 0 = h_-1
    = 0); no per-step h DMA; emission matmuls read it directly.
  - all nonlinearities are Sigmoid (tanh(x) = 2*sigmoid(2x) - 1); the
    device stores h' = h/2 and the host doubles W_hh and w_out to
    compensate, so a single activation-function table serves the whole
    loop. Gate math is fused into 3 scalar_tensor_tensor ops + one
    parallel GpSimd multiply:
       u = (sig(2g) - 0.5) * sig(i)         [DVE]   (= i*tanh(g)/2)
       v = sig(f) * c_old                   [GpSimd]
       c = 2*u + v                          [DVE]
       h' = (sig(2c) - 0.5) * sig(o)        [DVE]   (= h/2)

Device layouts (per core):
  xT      [768, 8192] bf16   col p = l*16+b, l = processing step (bwd cores
                             get time-reversed x so the device program is SPMD)
  wih_t   [128, 6*16*128]    stationary tiles (kc, m) of W_ih^T, g-block rows x2
  whh_t   [128, 4*16*128]    stationary tiles (k, m) of W_hh^T, g-rows x2, all x2
  bias    [128, 16] fp32     per-gate-tile bias (g-block x2)
  wo_t    [128, 4*9] bf16    stationary tiles of 2*w_out (this dir's 512 hid cols)
  bias_o  [9, 1] fp32        b_out on fwd cores, 0 on bwd cores
  ident   [128, 128] bf16    identity (stationary for the xp->PSUM inject)
  out: emisT [9, 8192] fp32  emission partial, col p = l*16+b (processing order)
"""

import numpy as np
import ml_dtypes

B, S, EMB = 64, 512, 768
HID = 512
NTAG = 9
BL = 16            # batch per core
NPOS = S * BL      # positions per core
KC_E = EMB // 128  # 6 k-chunks for projection
KC_H = HID // 128  # 4 k-chunks for recurrence
MT = 16            # gate tiles (4*HID/128)
UNROLL = 16
WHH_FP8 = True     # W_hh stationary in fp8e4m3: FWL loads 4 elem/cycle vs 2

_CACHED = {}


def _build_neff1():
    import concourse.bass as bass
    import concourse.bacc as bacc
    import concourse.mybir as mybir
    import concourse.tile as tile
    from concourse.bass import ds

    f32 = mybir.dt.float32
    bf16 = mybir.dt.bfloat16
    whh_dt = mybir.dt.float8e4 if WHH_FP8 else bf16

    nc = bacc.Bacc("TRN2", target_bir_lowering=False, debug=False)

    xT = nc.dram_tensor("xT", [EMB, NPOS], bf16, kind="ExternalInput")
    wih = nc.dram_tensor("wih", [128, KC_E * MT * 128], bf16, kind="ExternalInput")
    whh = nc.dram_tensor("whh", [128, KC_H * MT * 128], whh_dt, kind="ExternalInput")
    bias = nc.dram_tensor("bias", [128, MT], f32, kind="ExternalInput")
    wo = nc.dram_tensor("wo", [128, KC_H * NTAG], bf16, kind="ExternalInput")
    bias_o = nc.dram_tensor("bias_o", [NTAG, 1], f32, kind="ExternalInput")
    ident = nc.dram_tensor("ident", [128, 128], bf16, kind="ExternalInput")
    emisT = nc.dram_tensor("emisT", [NTAG, NPOS], f32, kind="ExternalOutput")

    # xp in DRAM, bf16, m-major: col = m*(S*BL) + s*BL + b
    xpT = nc.dram_tensor("xpT", [128, MT * S * BL], bf16)

    sig = mybir.ActivationFunctionType.Sigmoid
    mult = mybir.AluOpType.mult
    add = mybir.AluOpType.add
    subtract = mybir.AluOpType.subtract

    HB = KC_H * BL       # 64 cols: one gate-block (4 hid-chunks x 16 batch)
    BW = UNROLL * BL     # 128 cols per m-tile per body

    with tile.TileContext(nc) as tc:
        with (
            tc.tile_pool(name="wpool", bufs=1) as wpool,
            tc.tile_pool(name="x6pool", bufs=3) as x6pool,
            tc.tile_pool(name="xopool", bufs=3) as xopool,
            tc.tile_pool(name="xbpool", bufs=3) as xbpool,
            tc.tile_pool(name="gpool", bufs=3) as gpool,
            tc.tile_pool(name="spool", bufs=3) as spool,
            tc.tile_pool(name="eopool", bufs=3) as eopool,
            tc.tile_pool(name="pp1", bufs=2, space="PSUM") as pp1,
            tc.tile_pool(name="ppA", bufs=2, space="PSUM") as ppA,
            tc.tile_pool(name="ppB", bufs=2, space="PSUM") as ppB,
            tc.tile_pool(name="pp9", bufs=2, space="PSUM") as pp9,
        ):
            # --- resident weights ---
            wih_sb = wpool.tile([128, KC_E * MT * 128], bf16, tag="wih")
            whh_sb = wpool.tile([128, KC_H * MT * 128], whh_dt, tag="whh")
            bias_sb = wpool.tile([128, MT], f32, tag="bias")
            wo_sb = wpool.tile([128, KC_H * NTAG], bf16, tag="wo")
            bias_o_sb = wpool.tile([NTAG, 1], f32, tag="biaso")
            ident_sb = wpool.tile([128, 128], bf16, tag="ident")
            nc.sync.dma_start(out=wih_sb[:], in_=wih[:])
            nc.sync.dma_start(out=whh_sb[:], in_=whh[:])
            nc.sync.dma_start(out=bias_sb[:], in_=bias[:])
            nc.sync.dma_start(out=wo_sb[:], in_=wo[:])
            nc.sync.dma_start(out=bias_o_sb[:], in_=bias_o[:])
            nc.sync.dma_start(out=ident_sb[:], in_=ident[:])

            # --- persistent state ---
            # h ring (for emissions): slot s holds h'(s) = h(s)/2.
            # h_sb: static ping-pong read by the recurrence matmuls (register
            # offsets on matmul operands cost ~100ns/instruction on the PE
            # sequencer, so the hot loop must use static addresses).
            h_all = nc.alloc_sbuf_tensor("h_all", [128, S * HB], bf16).ap()
            h_sb = nc.alloc_sbuf_tensor("h_state", [128, 2 * HB], bf16).ap()
            c_sb = nc.alloc_sbuf_tensor("c_state", [128, 2 * HB], f32).ap()
            nc.vector.memset(h_sb[:], 0.0)
            nc.vector.memset(c_sb[:], 0.0)

            # ============================================================
            # Megakernel: straight-line code (no hardware loop). The 512
            # recurrence steps are emitted as 32 bodies of 16 steps with
            # static addresses; input-projection tasks (one (pc, m-tile)
            # GEMM each, 2 chunks of lookahead) and emission tasks are
            # interleaved into the PE idle gap of each step's serial
            # gate chain.
            # ============================================================
            BODY = UNROLL              # steps per body
            NB = S // BODY             # 32 bodies
            BPC = 32 // BODY           # bodies per 32-step xp chunk (2)
            xpT_m = xpT[:].rearrange("p (m sb) -> p m sb", m=MT)
            h_skb = h_all.rearrange("p (s k b) -> p s k b", k=KC_H, b=BL)

            xs6_tiles = {}

            def load_xs6(pc):
                xs6 = x6pool.tile([128, KC_E * 512], bf16, tag="xs6")
                for kc in range(KC_E):
                    nc.sync.dma_start(
                        out=xs6[:, kc * 512:(kc + 1) * 512],
                        in_=xT[kc * 128:(kc + 1) * 128, pc * 512:(pc + 1) * 512],
                    )
                xs6_tiles[pc] = xs6

            def proj_task(pc, m):
                # input-proj GEMM for one (32-step chunk, gate m-tile)
                if m == 0 and pc + 1 < MT:
                    load_xs6(pc + 1)
                xs6 = xs6_tiles[pc]
                ps = pp1.tile([128, 512], f32, tag="ppj")
                for kc in range(KC_E):
                    nc.tensor.matmul(
                        ps[:],
                        wih_sb[:, (kc * MT + m) * 128:(kc * MT + m) * 128 + 128],
                        xs6[:, kc * 512:(kc + 1) * 512],
                        start=(kc == 0),
                        stop=(kc == KC_E - 1),
                    )
                xo = xopool.tile([128, 512], bf16, tag="xo")
                nc.vector.tensor_scalar_add(xo[:], ps[:], bias_sb[:, m:m + 1])
                nc.sync.dma_start(
                    out=xpT[:, ds(m * (S * BL) + pc * 512, 512)], in_=xo[:])

            def emis_task(pc):
                ps9 = pp9.tile([NTAG, 512], f32, tag="ps9")
                for kc in range(KC_H):
                    nc.tensor.matmul(
                        ps9[:],
                        wo_sb[:, kc * NTAG:(kc + 1) * NTAG],
                        h_skb[:, pc * 32:(pc + 1) * 32, kc, :],
                        start=(kc == 0),
                        stop=(kc == KC_H - 1),
                    )
                eo = eopool.tile([NTAG, 512], f32, tag="eo")
                nc.vector.tensor_scalar_add(eo[:], ps9[:], bias_o_sb[:, 0:1])
                nc.sync.dma_start(out=emisT[:, pc * 512:(pc + 1) * 512], in_=eo[:])

            def load_xb(j):
                # xp tile for body j: [128, MT x BODY x BL] bf16, (m, i, b)
                xb = xbpool.tile([128, MT * BODY * BL], bf16, tag="xb")
                xbr = xb[:].rearrange("p (m ib) -> p m ib", m=MT)
                nc.sync.dma_start(
                    out=xbr[:, :, :],
                    in_=xpT_m[:, :, j * BODY * BL:(j + 1) * BODY * BL],
                )
                return xbr

            def step_body(xbr, iv, i, pi):
                po = 1 - pi
                psA = ppA.tile([128, 12 * BL], f32, tag="psA")
                psB = ppB.tile([128, 4 * BL], f32, tag="psB")

                # inject xp for i,f,g tiles; accumulate W_hh·h'(t-1)
                nc.tensor.matmul(
                    psA[:], ident_sb[:], xbr[:, 0:12, i * BL:(i + 1) * BL],
                    start=True, stop=False)
                for m in range(12):
                    for k in range(KC_H):
                        nc.tensor.matmul(
                            psA[:, m * BL:(m + 1) * BL],
                            whh_sb[:, (k * MT + m) * 128:(k * MT + m) * 128 + 128],
                            h_sb[:, pi * HB + k * BL:pi * HB + (k + 1) * BL],
                            start=False,
                            stop=(m == 11 and k == KC_H - 1),
                        )
                # o tiles in their own PSUM bank (no WAR with the sigmoid read)
                nc.tensor.matmul(
                    psB[:], ident_sb[:], xbr[:, 12:16, i * BL:(i + 1) * BL],
                    start=True, stop=False)
                for m in range(12, 16):
                    for k in range(KC_H):
                        nc.tensor.matmul(
                            psB[:, (m - 12) * BL:(m - 11) * BL],
                            whh_sb[:, (k * MT + m) * 128:(k * MT + m) * 128 + 128],
                            h_sb[:, pi * HB + k * BL:pi * HB + (k + 1) * BL],
                            start=False,
                            stop=(m == 15 and k == KC_H - 1),
                        )

                G = gpool.tile([128, 12 * BL], bf16, tag="G")
                nc.scalar.activation(G[:], psA[:], sig)
                Go = gpool.tile([128, HB], bf16, tag="Go")
                nc.scalar.activation(Go[:], psB[:], sig)

                # u = (sig2g - 0.5) * sigi ; v = sigf * c_old ; c = 2u + v
                u = spool.tile([128, HB], f32, tag="u")
                nc.vector.scalar_tensor_tensor(
                    u[:], G[:, 8 * BL:12 * BL], 0.5, G[:, 0:4 * BL],
                    subtract, mult)
                v = spool.tile([128, HB], f32, tag="v")
                nc.gpsimd.tensor_tensor(
                    out=v[:], in0=G[:, 4 * BL:8 * BL],
                    in1=c_sb[:, pi * HB:(pi + 1) * HB], op=mult)
                c_new = c_sb[:, po * HB:(po + 1) * HB]
                nc.vector.scalar_tensor_tensor(c_new, u[:], 2.0, v[:], mult, add)
                sc = gpool.tile([128, HB], bf16, tag="sc")
                nc.scalar.activation(sc[:], c_new, sig, scale=2.0)
                # h' = (sig(2c) - 0.5) * sig(o) = h/2
                h_new = h_sb[:, po * HB:(po + 1) * HB]
                nc.vector.scalar_tensor_tensor(
                    h_new, sc[:], 0.5, Go[:], subtract, mult)
                # mirror into the emission ring, off the critical path
                nc.gpsimd.tensor_copy(
                    out=h_all[:, iv * HB:(iv + 1) * HB], in_=h_new)

            # --- prologue: first two xp chunks (lookahead = 2) ---
            load_xs6(0)
            for m in range(MT):
                proj_task(0, m)
            for m in range(MT):
                proj_task(1, m)
            xbr_cur = load_xb(0)

            # --- main: bodies with interleaved proj/emission tasks ---
            # body j consumes chunk j//BPC; computes chunk j//BPC + 2
            tasks = [(pc, m) for pc in range(2, MT) for m in range(MT)]
            TPB = len(tasks) // 28 if NB >= 28 else 8  # 8 tasks/body
            for j in range(NB):
                if j + 1 < NB:
                    xbr_nxt = load_xb(j + 1)
                body_tasks = tasks[TPB * j:TPB * (j + 1)]
                emis_pc = (j - 2) // BPC if (j >= 2 and j % BPC == 0) else None
                for i in range(BODY):
                    iv = j * BODY + i
                    step_body(xbr_cur, iv, i, iv % 2)
                    # fill the PE gap after this step's matmul burst
                    if i % 2 == 1 and i // 2 < len(body_tasks):
                        proj_task(*body_tasks[i // 2])
                    if i == 0 and emis_pc is not None:
                        emis_task(emis_pc)
                if j + 1 < NB:
                    xbr_cur = xbr_nxt

            # --- epilogue: last emission chunk ---
            emis_task(MT - 1)

    nc.compile()
    return nc


def _prep_core_inputs(x, w_ih, w_hh, b_all, w_out, b_out, D, q):
    """Build the input dict for core (direction D, batch-quarter q)."""
    bf16 = ml_dtypes.bfloat16
    bs = slice(BL * q, BL * q + BL)
    xs = x[bs]                       # [16, S, EMB]
    if D == 1:
        xs = xs[:, ::-1, :]          # processing order = reversed time
    # xT[e, l*16+b] = xs[b, l, e]
    xT = np.ascontiguousarray(xs.transpose(2, 1, 0).reshape(EMB, NPOS)).astype(bf16)

    gscale = np.ones((4 * HID,), np.float32)
    gscale[2 * HID:3 * HID] = 2.0    # pytorch gate order i,f,g,o -> g block

    wihs = (w_ih * gscale[:, None]).astype(np.float32)   # [2048, 768]
    # extra x2: device stores h' = h/2
    whhs = (w_hh * gscale[:, None] * 2.0).astype(np.float32)   # [2048, 512]
    bs_ = (b_all * gscale).astype(np.float32)            # [2048]

    # wih tiles: [kr, (kc*MT+m)*128+mc] = wihs[m*128+mc, kc*128+kr]
    wt = wihs.reshape(MT, 128, KC_E, 128).transpose(3, 2, 0, 1)   # [kr, kc, m, mc]
    wih_t = np.ascontiguousarray(wt.reshape(128, KC_E * MT * 128)).astype(bf16)
    ht = whhs.reshape(MT, 128, KC_H, 128).transpose(3, 2, 0, 1)
    whh_np_dt = ml_dtypes.float8_e4m3 if WHH_FP8 else bf16
    whh_t = np.ascontiguousarray(ht.reshape(128, KC_H * MT * 128)).astype(whh_np_dt)
    bias_t = np.ascontiguousarray(bs_.reshape(MT, 128).T).astype(np.float32)

    # wo tiles: [kr, kc*9+t] = 2*w_out[t, D*512 + kc*128 + kr]  (h' = h/2)
    wo_half = 2.0 * w_out[:, D * HID:(D + 1) * HID]      # [9, 512]
    wo_t = np.ascontiguousarray(
        wo_half.reshape(NTAG, KC_H, 128).transpose(2, 1, 0).reshape(128, KC_H * NTAG)
    ).astype(bf16)
    bias_o = (b_out.reshape(NTAG, 1) if D == 0 else np.zeros((NTAG, 1))).astype(np.float32)
    ident_t = np.eye(128, dtype=np.float32).astype(bf16)

    return {
        "xT": np.asarray(xT), "wih": wih_t, "whh": whh_t, "bias": bias_t,
        "wo": wo_t, "bias_o": bias_o, "ident": ident_t,
    }


def _crf_loss_host(emis, tags, mask, start_trans, end_trans, trans):
    """emis [S, B, T] fp32 (time-major), tags [S, B], mask [S, B]. Exact numpy CRF."""
    Sq, Bq, T = emis.shape
    bidx = np.arange(Bq)
    m = mask.astype(np.float64)
    e = emis.astype(np.float64)
    tr = trans.astype(np.float64)
    num = start_trans.astype(np.float64)[tags[0]] + e[0, bidx, tags[0]]
    trans_steps = tr[tags[:-1], tags[1:]]
    emit_steps = np.take_along_axis(e[1:], tags[1:, :, None], axis=2)[..., 0]
    num = num + ((trans_steps + emit_steps) * m[1:]).sum(0)
    last_idx = m.sum(0).astype(np.int64) - 1
    num = num + end_trans.astype(np.float64)[tags[last_idx, bidx]]

    alpha = start_trans.astype(np.float64) + e[0]        # [B, T]
    for t in range(1, Sq):
        x = alpha[:, :, None] + tr[None] + e[t][:, None, :]
        mx = x.max(1)
        nxt = mx + np.log(np.exp(x - mx[:, None, :]).sum(1))
        alpha = np.where(m[t][:, None] > 0, nxt, alpha)
    z = alpha + end_trans.astype(np.float64)
    mz = z.max(1)
    den = mz + np.log(np.exp(z - mz[:, None]).sum(1))
    llh = num - den
    return -(llh.sum() / m.sum())


def kernel(x, mask, target_tag, w_ih_f, w_hh_f, b_f, w_ih_b, w_hh_b, b_b,
           w_out, b_out, start_trans, end_trans, trans):
    from concourse.bass_utils import run_bass_kernel_spmd

    x = np.asarray(x, np.float32)
    mask = np.asarray(mask)
    target_tag = np.asarray(target_tag)
    w_out = np.asarray(w_out, np.float32)
    b_out = np.asarray(b_out, np.float32)

    if "nc" not in _CACHED:
        _CACHED["nc"] = _build_neff1()
    nc = _CACHED["nc"]

    in_maps = []
    for core in range(8):
        D, q = core // 4, core % 4
        w_ih = np.asarray(w_ih_f if D == 0 else w_ih_b, np.float32)
        w_hh = np.asarray(w_hh_f if D == 0 else w_hh_b, np.float32)
        b_all = np.asarray(b_f if D == 0 else b_b, np.float32)
        in_maps.append(_prep_core_inputs(x, w_ih, w_hh, b_all, w_out, b_out, D, q))

    res = run_bass_kernel_spmd(nc, in_maps, core_ids=list(range(8)))

    # merge emissions: emis[s, b, t]
    emis = np.zeros((S, B, NTAG), np.float32)
    for core in range(8):
        D, q = core // 4, core % 4
        eT = res.results[core]["emisT"]                 # [9, S*16] processing order
        e = eT.reshape(NTAG, S, BL).transpose(1, 2, 0)  # [S(proc), 16, 9]
        if D == 1:
            e = e[::-1]
        emis[:, BL * q:BL * q + BL, :] += e

    loss = _crf_loss_host(
        emis, np.asarray(target_tag).T, np.asarray(mask).T.astype(np.float32),
        np.asarray(start_trans, np.float32), np.asarray(end_trans, np.float32),
        np.asarray(trans, np.float32),
    )
    return np.float32(loss)


# revision 16
# speedup vs baseline: 3.3666x; 1.0506x over previous
"""BiLSTM-CRF token-mean NLL loss on 8 Trainium2 NeuronCores.

Sharding: 8 cores = 2 LSTM directions x 4 batch-quarters (B_l=16).
Each core runs: input projection (x @ W_ih^T + b), the 512-step LSTM
recurrence for its direction, and its direction's half of the emission
projection. Host merges the two emission halves per batch-quarter and
computes the (tiny) CRF forward algorithm + gold-path score reduction.

v2 design notes (vs the v1 baseline at 3.81 ms):
  - input-proj xp is stored bf16 in DRAM with an m-major layout so the
    phase-1 writes are contiguous and the recurrence loads one 4KB/
    partition tile per 8-step body (prefetched; v1 stalled ~1.8us/step
    on a late per-step strided load).
  - xp is injected into PSUM with an identity-stationary matmul
    (start=True) so the W_hh matmuls accumulate on top of it; the two
    per-step DVE adds (gate preact = ps + xs) disappear and the sigmoid
    reads PSUM directly.
  - separate PSUM tiles for the i,f,g block and the o block remove a
    false WAR dependency (v1's o-gate matmuls waited on the gate add).
  - h lives entirely in SBUF ([128, (S+1)*64] bf16 ring, slot 0 = h_-1
    = 0); no per-step h DMA; emission matmuls read it directly.
  - all nonlinearities are Sigmoid (tanh(x) = 2*sigmoid(2x) - 1); the
    device stores h' = h/2 and the host doubles W_hh and w_out to
    compensate, so a single activation-function table serves the whole
    loop. Gate math is fused into 3 scalar_tensor_tensor ops + one
    parallel GpSimd multiply:
       u = (sig(2g) - 0.5) * sig(i)         [DVE]   (= i*tanh(g)/2)
       v = sig(f) * c_old                   [GpSimd]
       c = 2*u + v                          [DVE]
       h' = (sig(2c) - 0.5) * sig(o)        [DVE]   (= h/2)

Device layouts (per core):
  xT      [768, 8192] bf16   col p = l*16+b, l = processing step (bwd cores
                             get time-reversed x so the device program is SPMD)
  wih_t   [128, 6*16*128]    stationary tiles (kc, m) of W_ih^T, g-block rows x2
  whh_t   [128, 4*16*128]    stationary tiles (k, m) of W_hh^T, g-rows x2, all x2
  bias    [128, 16] fp32     per-gate-tile bias (g-block x2)
  wo_t    [128, 4*9] bf16    stationary tiles of 2*w_out (this dir's 512 hid cols)
  bias_o  [9, 1] fp32        b_out on fwd cores, 0 on bwd cores
  ident   [128, 128] bf16    identity (stationary for the xp->PSUM inject)
  out: emisT [9, 8192] fp32  emission partial, col p = l*16+b (processing order)
"""

import numpy as np
import ml_dtypes

B, S, EMB = 64, 512, 768
HID = 512
NTAG = 9
BL = 16            # batch per core
NPOS = S * BL      # positions per core
KC_E = EMB // 128  # 6 k-chunks for projection
KC_H = HID // 128  # 4 k-chunks for recurrence
MT = 16            # gate tiles (4*HID/128)
UNROLL = 16
WHH_FP8 = True     # W_hh stationary in fp8e4m3: FWL loads 4 elem/cycle vs 2

_CACHED = {}


def _build_neff1():
    import concourse.bass as bass
    import concourse.bacc as bacc
    import concourse.mybir as mybir
    import concourse.tile as tile
    from concourse.bass import ds

    f32 = mybir.dt.float32
    bf16 = mybir.dt.bfloat16
    whh_dt = mybir.dt.float8e4 if WHH_FP8 else bf16

    nc = bacc.Bacc("TRN2", target_bir_lowering=False, debug=False)

    xT = nc.dram_tensor("xT", [EMB, NPOS], bf16, kind="ExternalInput")
    wih = nc.dram_tensor("wih", [128, KC_E * MT * 128], bf16, kind="ExternalInput")
    whh = nc.dram_tensor("whh", [128, KC_H * MT * 128], whh_dt, kind="ExternalInput")
    bias = nc.dram_tensor("bias", [128, MT], f32, kind="ExternalInput")
    wo = nc.dram_tensor("wo", [128, KC_H * NTAG], bf16, kind="ExternalInput")
    bias_o = nc.dram_tensor("bias_o", [NTAG, 1], f32, kind="ExternalInput")
    ident = nc.dram_tensor("ident", [128, 128], bf16, kind="ExternalInput")
    emisT = nc.dram_tensor("emisT", [NTAG, NPOS], f32, kind="ExternalOutput")

    # xp in DRAM, bf16, m-major: col = m*(S*BL) + s*BL + b
    xpT = nc.dram_tensor("xpT", [128, MT * S * BL], bf16)

    sig = mybir.ActivationFunctionType.Sigmoid
    mult = mybir.AluOpType.mult
    add = mybir.AluOpType.add
    subtract = mybir.AluOpType.subtract

    HB = KC_H * BL       # 64 cols: one gate-block (4 hid-chunks x 16 batch)
    BW = UNROLL * BL     # 128 cols per m-tile per body

    with tile.TileContext(nc) as tc:
        with (
            tc.tile_pool(name="wpool", bufs=1) as wpool,
            tc.tile_pool(name="x6pool", bufs=3) as x6pool,
            tc.tile_pool(name="xopool", bufs=3) as xopool,
            tc.tile_pool(name="xbpool", bufs=3) as xbpool,
            tc.tile_pool(name="gpool", bufs=3) as gpool,
            tc.tile_pool(name="spool", bufs=3) as spool,
            tc.tile_pool(name="eopool", bufs=3) as eopool,
            tc.tile_pool(name="pp1", bufs=2, space="PSUM") as pp1,
            tc.tile_pool(name="ppIF", bufs=2, space="PSUM") as ppIF,
            tc.tile_pool(name="ppG", bufs=2, space="PSUM") as ppG,
            tc.tile_pool(name="ppB", bufs=1, space="PSUM") as ppB,
            tc.tile_pool(name="pp9", bufs=1, space="PSUM") as pp9,
        ):
            # --- resident weights ---
            wih_sb = wpool.tile([128, KC_E * MT * 128], bf16, tag="wih")
            whh_sb = wpool.tile([128, KC_H * MT * 128], whh_dt, tag="whh")
            bias_sb = wpool.tile([128, MT], f32, tag="bias")
            wo_sb = wpool.tile([128, KC_H * NTAG], bf16, tag="wo")
            bias_o_sb = wpool.tile([NTAG, 1], f32, tag="biaso")
            ident_sb = wpool.tile([128, 128], bf16, tag="ident")
            nc.sync.dma_start(out=wih_sb[:], in_=wih[:])
            nc.sync.dma_start(out=whh_sb[:], in_=whh[:])
            nc.sync.dma_start(out=bias_sb[:], in_=bias[:])
            nc.sync.dma_start(out=wo_sb[:], in_=wo[:])
            nc.sync.dma_start(out=bias_o_sb[:], in_=bias_o[:])
            nc.sync.dma_start(out=ident_sb[:], in_=ident[:])

            # --- persistent state ---
            # h ring (for emissions): slot s holds h'(s) = h(s)/2.
            # h_sb: static ping-pong read by the recurrence matmuls (register
            # offsets on matmul operands cost ~100ns/instruction on the PE
            # sequencer, so the hot loop must use static addresses).
            h_all = nc.alloc_sbuf_tensor("h_all", [128, S * HB], bf16).ap()
            h_sb = nc.alloc_sbuf_tensor("h_state", [128, 2 * HB], bf16).ap()
            c_sb = nc.alloc_sbuf_tensor("c_state", [128, 2 * HB], f32).ap()
            nc.vector.memset(h_sb[:], 0.0)
            nc.vector.memset(c_sb[:], 0.0)

            # ============================================================
            # Megakernel: straight-line code (no hardware loop). The 512
            # recurrence steps are emitted as 32 bodies of 16 steps with
            # static addresses; input-projection tasks (one (pc, m-tile)
            # GEMM each, 2 chunks of lookahead) and emission tasks are
            # interleaved into the PE idle gap of each step's serial
            # gate chain.
            # ============================================================
            BODY = UNROLL              # steps per body
            NB = S // BODY             # 32 bodies
            BPC = 32 // BODY           # bodies per 32-step xp chunk (2)
            xpT_m = xpT[:].rearrange("p (m sb) -> p m sb", m=MT)
            h_skb = h_all.rearrange("p (s k b) -> p s k b", k=KC_H, b=BL)

            xs6_tiles = {}

            def load_xs6(pc):
                xs6 = x6pool.tile([128, KC_E * 512], bf16, tag="xs6")
                for kc in range(KC_E):
                    nc.sync.dma_start(
                        out=xs6[:, kc * 512:(kc + 1) * 512],
                        in_=xT[kc * 128:(kc + 1) * 128, pc * 512:(pc + 1) * 512],
                    )
                xs6_tiles[pc] = xs6

            def proj_task(pc, m):
                # input-proj GEMM for one (32-step chunk, gate m-tile)
                if m == 0 and pc + 1 < MT:
                    load_xs6(pc + 1)
                xs6 = xs6_tiles[pc]
                ps = pp1.tile([128, 512], f32, tag="ppj")
                for kc in range(KC_E):
                    nc.tensor.matmul(
                        ps[:],
                        wih_sb[:, (kc * MT + m) * 128:(kc * MT + m) * 128 + 128],
                        xs6[:, kc * 512:(kc + 1) * 512],
                        start=(kc == 0),
                        stop=(kc == KC_E - 1),
                    )
                xo = xopool.tile([128, 512], bf16, tag="xo")
                nc.vector.tensor_scalar_add(xo[:], ps[:], bias_sb[:, m:m + 1])
                nc.sync.dma_start(
                    out=xpT[:, ds(m * (S * BL) + pc * 512, 512)], in_=xo[:])

            def emis_task(pc):
                ps9 = pp9.tile([NTAG, 512], f32, tag="ps9")
                for kc in range(KC_H):
                    nc.tensor.matmul(
                        ps9[:],
                        wo_sb[:, kc * NTAG:(kc + 1) * NTAG],
                        h_skb[:, pc * 32:(pc + 1) * 32, kc, :],
                        start=(kc == 0),
                        stop=(kc == KC_H - 1),
                    )
                eo = eopool.tile([NTAG, 512], f32, tag="eo")
                nc.vector.tensor_scalar_add(eo[:], ps9[:], bias_o_sb[:, 0:1])
                nc.sync.dma_start(out=emisT[:, pc * 512:(pc + 1) * 512], in_=eo[:])

            def load_xb(j):
                # xp tile for body j: [128, MT x BODY x BL] bf16, (m, i, b)
                xb = xbpool.tile([128, MT * BODY * BL], bf16, tag="xb")
                xbr = xb[:].rearrange("p (m ib) -> p m ib", m=MT)
                nc.sync.dma_start(
                    out=xbr[:, :, :],
                    in_=xpT_m[:, :, j * BODY * BL:(j + 1) * BODY * BL],
                )
                return xbr

            def step_body(xbr, iv, i, pi):
                po = 1 - pi
                psIF = ppIF.tile([128, 8 * BL], f32, tag="psIF")
                psG = ppG.tile([128, 4 * BL], f32, tag="psG")
                psB = ppB.tile([128, 4 * BL], f32, tag="psB")

                # i,f tiles first: their sigmoid runs while g/o matmuls finish
                nc.tensor.matmul(
                    psIF[:], ident_sb[:], xbr[:, 0:8, i * BL:(i + 1) * BL],
                    start=True, stop=False)
                for m in range(8):
                    for k in range(KC_H):
                        nc.tensor.matmul(
                            psIF[:, m * BL:(m + 1) * BL],
                            whh_sb[:, (k * MT + m) * 128:(k * MT + m) * 128 + 128],
                            h_sb[:, pi * HB + k * BL:pi * HB + (k + 1) * BL],
                            start=False,
                            stop=(m == 7 and k == KC_H - 1),
                        )
                nc.tensor.matmul(
                    psG[:], ident_sb[:], xbr[:, 8:12, i * BL:(i + 1) * BL],
                    start=True, stop=False)
                for m in range(8, 12):
                    for k in range(KC_H):
                        nc.tensor.matmul(
                            psG[:, (m - 8) * BL:(m - 7) * BL],
                            whh_sb[:, (k * MT + m) * 128:(k * MT + m) * 128 + 128],
                            h_sb[:, pi * HB + k * BL:pi * HB + (k + 1) * BL],
                            start=False,
                            stop=(m == 11 and k == KC_H - 1),
                        )
                nc.tensor.matmul(
                    psB[:], ident_sb[:], xbr[:, 12:16, i * BL:(i + 1) * BL],
                    start=True, stop=False)
                for m in range(12, 16):
                    for k in range(KC_H):
                        nc.tensor.matmul(
                            psB[:, (m - 12) * BL:(m - 11) * BL],
                            whh_sb[:, (k * MT + m) * 128:(k * MT + m) * 128 + 128],
                            h_sb[:, pi * HB + k * BL:pi * HB + (k + 1) * BL],
                            start=False,
                            stop=(m == 15 and k == KC_H - 1),
                        )

                Gif = gpool.tile([128, 8 * BL], bf16, tag="Gif")
                nc.scalar.activation(Gif[:], psIF[:], sig)
                Gg = gpool.tile([128, HB], bf16, tag="Gg")
                nc.scalar.activation(Gg[:], psG[:], sig)
                Go = gpool.tile([128, HB], bf16, tag="Go")
                nc.scalar.activation(Go[:], psB[:], sig)

                # u = (sig2g - 0.5) * sigi ; v = sigf * c_old ; c = 2u + v
                v = spool.tile([128, HB], f32, tag="v")
                nc.gpsimd.tensor_tensor(
                    out=v[:], in0=Gif[:, 4 * BL:8 * BL],
                    in1=c_sb[:, pi * HB:(pi + 1) * HB], op=mult)
                u = spool.tile([128, HB], f32, tag="u")
                nc.vector.scalar_tensor_tensor(
                    u[:], Gg[:], 0.5, Gif[:, 0:4 * BL],
                    subtract, mult)
                c_new = c_sb[:, po * HB:(po + 1) * HB]
                nc.vector.scalar_tensor_tensor(c_new, u[:], 2.0, v[:], mult, add)
                sc = gpool.tile([128, HB], bf16, tag="sc")
                nc.scalar.activation(sc[:], c_new, sig, scale=2.0)
                # h' = (sig(2c) - 0.5) * sig(o) = h/2
                h_new = h_sb[:, po * HB:(po + 1) * HB]
                nc.vector.scalar_tensor_tensor(
                    h_new, sc[:], 0.5, Go[:], subtract, mult)
                # mirror into the emission ring, off the critical path
                nc.gpsimd.tensor_copy(
                    out=h_all[:, iv * HB:(iv + 1) * HB], in_=h_new)

            # --- prologue: first two xp chunks (lookahead = 2) ---
            load_xs6(0)
            for m in range(MT):
                proj_task(0, m)
            for m in range(MT):
                proj_task(1, m)
            xbr_cur = load_xb(0)

            # --- main: bodies with interleaved proj/emission tasks ---
            # body j consumes chunk j//BPC; computes chunk j//BPC + 2
            tasks = [(pc, m) for pc in range(2, MT) for m in range(MT)]
            TPB = len(tasks) // 28 if NB >= 28 else 8  # 8 tasks/body
            for j in range(NB):
                if j + 1 < NB:
                    xbr_nxt = load_xb(j + 1)
                body_tasks = tasks[TPB * j:TPB * (j + 1)]
                emis_pc = (j - 2) // BPC if (j >= 2 and j % BPC == 0) else None
                for i in range(BODY):
                    iv = j * BODY + i
                    step_body(xbr_cur, iv, i, iv % 2)
                    # fill the PE gap after this step's matmul burst
                    if i % 2 == 1 and i // 2 < len(body_tasks):
                        proj_task(*body_tasks[i // 2])
                    if i == 0 and emis_pc is not None:
                        emis_task(emis_pc)
                if j + 1 < NB:
                    xbr_cur = xbr_nxt

            # --- epilogue: last emission chunk ---
            emis_task(MT - 1)

    nc.compile()
    return nc


def _prep_core_inputs(x, w_ih, w_hh, b_all, w_out, b_out, D, q):
    """Build the input dict for core (direction D, batch-quarter q)."""
    bf16 = ml_dtypes.bfloat16
    bs = slice(BL * q, BL * q + BL)
    xs = x[bs]                       # [16, S, EMB]
    if D == 1:
        xs = xs[:, ::-1, :]          # processing order = reversed time
    # xT[e, l*16+b] = xs[b, l, e]
    xT = np.ascontiguousarray(xs.transpose(2, 1, 0).reshape(EMB, NPOS)).astype(bf16)

    gscale = np.ones((4 * HID,), np.float32)
    gscale[2 * HID:3 * HID] = 2.0    # pytorch gate order i,f,g,o -> g block

    wihs = (w_ih * gscale[:, None]).astype(np.float32)   # [2048, 768]
    # extra x2: device stores h' = h/2
    whhs = (w_hh * gscale[:, None] * 2.0).astype(np.float32)   # [2048, 512]
    bs_ = (b_all * gscale).astype(np.float32)            # [2048]

    # wih tiles: [kr, (kc*MT+m)*128+mc] = wihs[m*128+mc, kc*128+kr]
    wt = wihs.reshape(MT, 128, KC_E, 128).transpose(3, 2, 0, 1)   # [kr, kc, m, mc]
    wih_t = np.ascontiguousarray(wt.reshape(128, KC_E * MT * 128)).astype(bf16)
    ht = whhs.reshape(MT, 128, KC_H, 128).transpose(3, 2, 0, 1)
    whh_np_dt = ml_dtypes.float8_e4m3 if WHH_FP8 else bf16
    whh_t = np.ascontiguousarray(ht.reshape(128, KC_H * MT * 128)).astype(whh_np_dt)
    bias_t = np.ascontiguousarray(bs_.reshape(MT, 128).T).astype(np.float32)

    # wo tiles: [kr, kc*9+t] = 2*w_out[t, D*512 + kc*128 + kr]  (h' = h/2)
    wo_half = 2.0 * w_out[:, D * HID:(D + 1) * HID]      # [9, 512]
    wo_t = np.ascontiguousarray(
        wo_half.reshape(NTAG, KC_H, 128).transpose(2, 1, 0).reshape(128, KC_H * NTAG)
    ).astype(bf16)
    bias_o = (b_out.reshape(NTAG, 1) if D == 0 else np.zeros((NTAG, 1))).astype(np.float32)
    ident_t = np.eye(128, dtype=np.float32).astype(bf16)

    return {
        "xT": np.asarray(xT), "wih": wih_t, "whh": whh_t, "bias": bias_t,
        "wo": wo_t, "bias_o": bias_o, "ident": ident_t,
    }


def _crf_loss_host(emis, tags, mask, start_trans, end_trans, trans):
    """emis [S, B, T] fp32 (time-major), tags [S, B], mask [S, B]. Exact numpy CRF."""
    Sq, Bq, T = emis.shape
    bidx = np.arange(Bq)
    m = mask.astype(np.float64)
    e = emis.astype(np.float64)
    tr = trans.astype(np.float64)
    num = start_trans.astype(np.float64)[tags[0]] + e[0, bidx, tags[0]]
    trans_steps = tr[tags[:-1], tags[1:]]
    emit_steps = np.take_along_axis(e[1:], tags[1:, :, None], axis=2)[..., 0]
    num = num + ((trans_steps + emit_steps) * m[1:]).sum(0)
    last_idx = m.sum(0).astype(np.int64) - 1
    num = num + end_trans.astype(np.float64)[tags[last_idx, bidx]]

    alpha = start_trans.astype(np.float64) + e[0]        # [B, T]
    for t in range(1, Sq):
        x = alpha[:, :, None] + tr[None] + e[t][:, None, :]
        mx = x.max(1)
        nxt = mx + np.log(np.exp(x - mx[:, None, :]).sum(1))
        alpha = np.where(m[t][:, None] > 0, nxt, alpha)
    z = alpha + end_trans.astype(np.float64)
    mz = z.max(1)
    den = mz + np.log(np.exp(z - mz[:, None]).sum(1))
    llh = num - den
    return -(llh.sum() / m.sum())


def kernel(x, mask, target_tag, w_ih_f, w_hh_f, b_f, w_ih_b, w_hh_b, b_b,
           w_out, b_out, start_trans, end_trans, trans):
    from concourse.bass_utils import run_bass_kernel_spmd

    x = np.asarray(x, np.float32)
    mask = np.asarray(mask)
    target_tag = np.asarray(target_tag)
    w_out = np.asarray(w_out, np.float32)
    b_out = np.asarray(b_out, np.float32)

    if "nc" not in _CACHED:
        _CACHED["nc"] = _build_neff1()
    nc = _CACHED["nc"]

    in_maps = []
    for core in range(8):
        D, q = core // 4, core % 4
        w_ih = np.asarray(w_ih_f if D == 0 else w_ih_b, np.float32)
        w_hh = np.asarray(w_hh_f if D == 0 else w_hh_b, np.float32)
        b_all = np.asarray(b_f if D == 0 else b_b, np.float32)
        in_maps.append(_prep_core_inputs(x, w_ih, w_hh, b_all, w_out, b_out, D, q))

    res = run_bass_kernel_spmd(nc, in_maps, core_ids=list(range(8)))

    # merge emissions: emis[s, b, t]
    emis = np.zeros((S, B, NTAG), np.float32)
    for core in range(8):
        D, q = core // 4, core % 4
        eT = res.results[core]["emisT"]                 # [9, S*16] processing order
        e = eT.reshape(NTAG, S, BL).transpose(1, 2, 0)  # [S(proc), 16, 9]
        if D == 1:
            e = e[::-1]
        emis[:, BL * q:BL * q + BL, :] += e

    loss = _crf_loss_host(
        emis, np.asarray(target_tag).T, np.asarray(mask).T.astype(np.float32),
        np.asarray(start_trans, np.float32), np.asarray(end_trans, np.float32),
        np.asarray(trans, np.float32),
    )
    return np.float32(loss)
